# revision 63
# baseline (speedup 1.0000x reference)
"""Trainium2 Bass kernel for nn_BiasedLoss: mean(|x * t|) with per-row argmax masking.

Reference semantics (x: [N,C] f32, target: [N,C] f32 in {0,1}):
    idx  = argmax(x, axis=1)
    cond = (idx > 0) & (target[:, 0] == 0)
    t    = where(cond, target * one_hot(idx), target)
    out  = mean(|x * t|)

Host-side encoding (lossy but well within the 2e-2 tolerance; measured
rel-err ~2e-4): x is rounded to the bf16-with-even-LSB grid (15 bits,
shifted half an ulp16 when t=1 so the write-back stays centered) and
target's 0/1 bit is stored in the mantissa LSB:

    enc = round15(x - t*ulp16) << 1 | t     (uint16 holding bf16 bits)

This collapses the two 16 MiB input streams into one 8 MiB stream and turns
the whole per-row computation into cheap ops:

    m    = max_c enc[r, c]   (bf16 float max; the LSB tie-breaks toward t=1)
    t_at = m & 1             (target at argmax == LSB of the row max)
    |m|*t_at = m & ((m & 1) * 0x7FFF)
    cond = (enc[r,0] < m) & (enc[r,0] & 1 == 0)
    |p|  = enc & ((enc & 1) * 0x7FFF)       (per-element |x*t|, sign cleared)
    out  = sum_r [ cond ? |m|*t_at : sum_c |p|[r, c] ] / (N*C)

TRN2 ISA constraints honored (walrus-verified): bitwise ops are DVE-only and
single-op (no mixed bitwise/arith fusion, no mod, no shifts); Pool has no
max/bitwise/int ops, only float mult/add/sub; ACT is single-input activations.

Device mapping per chunk ([128 partitions, segs x 128 cols], sizes tapered):
    DVE   : t01 = enc & 1 (ts); row-max as a tensor_tensor max tree (2x)
            + 8-wide reduce; slot blends.
            DVE-chunks: am7 = t01 * 0x7FFF (int ts), pa = enc & am7 (tt).
            GP-chunks:  t01f = t01 * bits(1.0bf16) (int ts).
    GPSIMD: GP-chunks: ps = enc * t01f  (float mult -> signed x*t)
    ACT   : GP-chunks: pa = Abs(ps); x0 strided extract (all chunks)
    PE    : per-segment matmul psum[128,1] += pa_seg^T @ w_slot with
            w = 1-cond; stationary operand is the data tile, moving operand
            one column, so each matmul is ~1 cycle: the whole weighted
            row-sum is free.
Emission is stage-zipped across chunk pairs so serial same-engine dependency
chains always have independent work between them (hides sem latency).

Sharding: pure data-parallel over the batch dim, 8 cores, 32768 rows each.
Host sums the [128,1] per-core outputs and divides by N*C.
"""

import numpy as np

N, C = 262144, 128
N_CORES = 8
ROWS_PER_CORE = N // N_CORES   # 32768

# chunk sizes (rows): big-early so the Pool backlog front-loads, small-late
# for a fast pipeline drain; pairs are zipped in the pipeline
CHUNKS = [1024, 2048, 4096, 4096, 4096, 4096, 4096, 4096, 2048, 1536, 1024, 512]
assert sum(CHUNKS) == ROWS_PER_CORE and len(CHUNKS) % 2 == 0
S_TOT = ROWS_PER_CORE // C     # stat slots per partition (256)

# per-chunk fraction of segments whose |x*t| is produced by the GPSIMD
# float-mult + ACT abs path instead of the DVE bitwise chain.  Front-loaded:
# Pool's serial backlog starts at chunk 0 and drains before the kernel tail,
# while the late chunks take the (cheaper-overall) DVE path.
GP_FRACS = (1.0, 1.0, 0.9, 0.85, 0.8, 0.7, 0.6, 0.45, 0.25, 0.0, 0.0, 0.0)

_cache = {}


def _build_nc():
    import concourse.bacc as bacc
    from concourse import mybir
    from concourse import tile as tile_mod

    f32 = mybir.dt.float32
    bf16 = mybir.dt.bfloat16
    u16 = mybir.dt.uint16
    A = mybir.AluOpType
    X = mybir.AxisListType.X
    AF = mybir.ActivationFunctionType

    n_ch = len(CHUNKS)
    base = [0] * n_ch
    for i in range(1, n_ch):
        base[i] = base[i - 1] + CHUNKS[i - 1]

    nc = bacc.Bacc("TRN2", target_bir_lowering=False, debug=False)

    x_d = nc.dram_tensor("x", [ROWS_PER_CORE, C], u16, kind="ExternalInput")
    out_d = nc.dram_tensor("out", [128, 1], f32, kind="ExternalOutput")

    with tile_mod.TileContext(nc) as tc:
        with (
            tc.tile_pool(name="enc", bufs=7) as enc_pool,
            tc.tile_pool(name="t01", bufs=8) as t01_pool,
            tc.tile_pool(name="pa", bufs=8) as pa_pool,
            tc.tile_pool(name="scr", bufs=2) as scr_pool,
            tc.tile_pool(name="stats", bufs=1) as stat_pool,
            tc.tile_pool(name="ps", bufs=1, space="PSUM") as psum_pool,
        ):
            m_all = stat_pool.tile([128, S_TOT], bf16, name="m_all")
            x0_all = stat_pool.tile([128, S_TOT], u16, name="x0_all")
            w_all = stat_pool.tile([128, S_TOT], bf16, name="w_all")
            term2 = stat_pool.tile([128, S_TOT], f32, name="term2")
            psum = psum_pool.tile([128, 1], f32, name="psum")

            m_u = m_all[:].bitcast(u16)
            x0_b = x0_all[:].bitcast(bf16)

            st = {}  # per-chunk tiles/views

            def alloc(ci):
                rows = CHUNKS[ci]
                enc = enc_pool.tile([128, rows], u16, tag="enc", name=f"enc{ci}")
                t01 = t01_pool.tile([128, rows], u16, tag="t01", name=f"t01_{ci}")
                pa = pa_pool.tile([128, rows], u16, tag="pa", name=f"pa{ci}")
                st[ci] = {
                    "enc": enc,
                    "t01": t01,
                    "pa": pa,
                    "encb": enc[:].bitcast(bf16).rearrange("p (s c) -> p s c", c=C),
                    "pab": pa[:].bitcast(bf16).rearrange("p (s c) -> p s c", c=C),
                }
                if ci % 2 == 0:
                    # one max-tree scratch tile per PAIR: levels below L1 run
                    # as single merged instructions over both chunks
                    prows = rows + CHUNKS[ci + 1]
                    tr = scr_pool.tile([128, prows // 2], u16, tag="tr",
                                       name=f"tr{ci}")
                    st[ci]["trb"] = tr[:].bitcast(bf16).rearrange(
                        "p (s c) -> p s c", c=C // 2
                    )

            def dma(ci):
                r0, rows = base[ci], CHUNKS[ci]
                src = x_d[r0 : r0 + rows, :].rearrange("(p s) c -> p (s c)", p=128)
                nc.sync.dma_start(out=st[ci]["enc"][:], in_=src)

            def gsplit(ci):  # elements routed via the GP path (seg multiple)
                return 128 * round(GP_FRACS[ci] * (CHUNKS[ci] // C))

            def t01(ci):  # t01 = enc & 1  (DVE ts, 4x)
                nc.vector.tensor_scalar(
                    out=st[ci]["t01"][:], in0=st[ci]["enc"][:],
                    scalar1=1, scalar2=None, op0=A.bitwise_and,
                )

            def conv(ci):  # t01f = float(t01) over the GP range
                ge = gsplit(ci)
                if not ge:
                    return
                if ci < 2:
                    # startup chunks: DVE int-mult (DVE is idle during ramp,
                    # and Pool must not wait for the ACT feed chain)
                    nc.vector.tensor_scalar(
                        out=st[ci]["t01"][:, 0:ge], in0=st[ci]["t01"][:, 0:ge],
                        scalar1=0x3F80, scalar2=None, op0=A.mult,
                    )
                else:
                    nc.scalar.activation(
                        out=st[ci]["t01"][:, 0:ge].bitcast(bf16),
                        in_=st[ci]["t01"][:, 0:ge], func=AF.Copy,
                    )

            def gate_dv(ci):  # am7 = t01 * 0x7FFF over the DVE range
                ge, rows = gsplit(ci), CHUNKS[ci]
                if ge < rows:
                    nc.vector.tensor_scalar(
                        out=st[ci]["t01"][:, ge:rows],
                        in0=st[ci]["t01"][:, ge:rows],
                        scalar1=0x7FFF, scalar2=None, op0=A.mult,
                    )

            def absenc(ci):  # pa[0:ge] = |enc|  (upstream of Pool)
                ge = gsplit(ci)
                if not ge:
                    return
                if ci < 2:
                    nc.vector.tensor_scalar(
                        out=st[ci]["pa"][:, 0:ge], in0=st[ci]["enc"][:, 0:ge],
                        scalar1=0x7FFF, scalar2=None, op0=A.bitwise_and,
                    )
                else:
                    nc.scalar.activation(
                        out=st[ci]["pa"][:, 0:ge].bitcast(bf16),
                        in_=st[ci]["enc"][:, 0:ge].bitcast(bf16), func=AF.Abs,
                    )

            def pa_gp(ci):  # pa[0:ge] *= t01f in place (Pool float mult)
                ge = gsplit(ci)
                if ge:
                    pb = st[ci]["pa"][:, 0:ge].bitcast(bf16)
                    nc.gpsimd.tensor_tensor(
                        out=pb, in0=pb,
                        in1=st[ci]["t01"][:, 0:ge].bitcast(bf16),
                        op=A.mult,
                    )

            def pa_dv(ci):  # pa = enc & am7 (DVE tt; mask also clears sign)
                ge, rows = gsplit(ci), CHUNKS[ci]
                if ge < rows:
                    nc.vector.tensor_tensor(
                        out=st[ci]["pa"][:, ge:rows],
                        in0=st[ci]["enc"][:, ge:rows],
                        in1=st[ci]["t01"][:, ge:rows], op=A.bitwise_and,
                    )

            def lvl1(ci):  # L1 for one chunk into the pair scratch
                pa_ci = ci - (ci % 2)
                off = 0 if ci % 2 == 0 else CHUNKS[pa_ci] // C
                segs = CHUNKS[ci] // C
                trb = st[pa_ci]["trb"]
                encb = st[ci]["encb"]
                nc.vector.tensor_tensor(
                    out=trb[:, off : off + segs, 0:64], in0=encb[:, :, 0:64],
                    in1=encb[:, :, 64:128], op=A.max,
                )

            def lvlm(pa_ci, n):  # merged level over both chunks of the pair
                trb = st[pa_ci]["trb"]
                nc.vector.tensor_tensor(
                    out=trb[:, :, 0:n], in0=trb[:, :, 0:n],
                    in1=trb[:, :, n : 2 * n], op=A.max,
                )

            def redm(pa_ci):  # merged 8-wide reduce into contiguous slots
                s0 = base[pa_ci] // C
                segs = (CHUNKS[pa_ci] + CHUNKS[pa_ci + 1]) // C
                nc.vector.tensor_reduce(
                    out=m_all[:, s0 : s0 + segs],
                    in_=st[pa_ci]["trb"][:, :, 0:8], axis=X, op=A.max,
                )

            def x0(ci):  # strided column-0 extract on ACT (bit-exact for bf16)
                s0 = base[ci] // C
                nc.scalar.activation(
                    out=x0_b[:, s0 : s0 + CHUNKS[ci] // C],
                    in_=st[ci]["encb"][:, :, 0], func=AF.Copy,
                )

            def blend_thunks(lo, hi):
                """w = 1-cond, term2 = cond * |m| * t_at over slots [lo,hi).
                Returns DVE-op thunks (in dependency order) for weaving into
                the pair schedule."""
                wd = hi - lo
                m_v = m_all[:, lo:hi]
                mu_v = m_u[:, lo:hi]
                x0u_v = x0_all[:, lo:hi]
                x0b_v = x0_b[:, lo:hi]

                def t2(name, dt=bf16):
                    return stat_pool.tile([128, wd], dt, name=f"{name}_{lo}")

                mt = t2("mt", u16)
                c1 = t2("c1")
                t0f = t2("t0f", u16)
                cond = t2("cond")
                return [
                    # |m|*t_at == m & ((m & 1) * 0x7FFF)  (single-op ts chain)
                    lambda: nc.vector.tensor_scalar(
                        out=mt[:], in0=mu_v, scalar1=1, scalar2=None,
                        op0=A.bitwise_and),
                    # c1 = x0 < m  (argmax > 0)
                    lambda: nc.vector.tensor_tensor(
                        out=c1[:], in0=x0b_v, in1=m_v, op=A.is_lt),
                    lambda: nc.vector.tensor_scalar(
                        out=mt[:], in0=mt[:], scalar1=0x7FFF, scalar2=None,
                        op0=A.mult),
                    # t0f = (x0 & 1) * bits(1.0bf16)  -> {0.0, 1.0}
                    lambda: nc.vector.tensor_scalar(
                        out=t0f[:], in0=x0u_v, scalar1=1, scalar2=None,
                        op0=A.bitwise_and),
                    lambda: nc.vector.tensor_tensor(
                        out=mt[:], in0=mt[:], in1=mu_v, op=A.bitwise_and),
                    lambda: nc.vector.tensor_scalar(
                        out=t0f[:], in0=t0f[:], scalar1=0x3F80, scalar2=None,
                        op0=A.mult),
                    # cond = c1 & !t0f == (c1 > t0f) on {0,1} floats
                    lambda: nc.vector.tensor_tensor(
                        out=cond[:], in0=c1[:], in1=t0f[:].bitcast(bf16),
                        op=A.is_gt),
                    # w = 1 - cond  (fused mult+add, same arith class)
                    lambda: nc.vector.tensor_scalar(
                        out=w_all[:, lo:hi], in0=cond[:], scalar1=-1.0,
                        scalar2=1.0, op0=A.mult, op1=A.add),
                    # term2 = cond * (|m|*t_at)
                    lambda: nc.vector.tensor_tensor(
                        out=term2[:, lo:hi], in0=cond[:],
                        in1=mt[:].bitcast(bf16), op=A.mult),
                ]

            mm_idx = [0]
            total_mms = S_TOT

            def pe(ci):
                pab = st[ci]["pab"]
                s0 = base[ci] // C
                for s in range(CHUNKS[ci] // C):
                    slot = s0 + s
                    nc.tensor.matmul(
                        psum[:, :],
                        pab[:, s, :],
                        w_all[:, slot : slot + 1],
                        start=(mm_idx[0] == 0),
                        stop=(mm_idx[0] == total_mms - 1),
                    )
                    mm_idx[0] += 1

            # --- stage-zipped schedule over chunk pairs -----------------
            for ci in range(4):
                alloc(ci)
                dma(ci)
            for k in range(0, n_ch, 2):
                a, b = k, k + 1
                if a + 4 < n_ch:
                    alloc(a + 4)
                    alloc(b + 4)
                    dma(a + 4)
                    dma(b + 4)
                # The GP-feeding chain (t01 -> ACT conv/abs -> Pool mult)
                # comes first so the Pool backlog starts immediately; ACT is
                # only UPSTREAM of Pool (no dependency ring).  The previous
                # pair's blend ops are woven between the merged tree levels
                # so every serial DVE dependency has independent work between
                # it (hides sem latency).
                B = blend_thunks(base[a - 2] // C, base[a] // C) if a >= 2 \
                    else []
                t01(a)
                t01(b)
                conv(a)
                absenc(a)
                conv(b)
                absenc(b)
                # Pool consumes the PREVIOUS pair's prepared tiles: one pair
                # of buffering between the ACT feed and the Pool mult; the
                # DVE-fed startup chunks 0/1 are multiplied without the lag
                if a == 0:
                    pa_gp(0)
                lvl1(a)
                if a == 0:
                    pa_gp(1)
                elif a >= 4:
                    pa_gp(a - 2)
                lvl1(b)
                if a >= 4:
                    pa_gp(b - 2)
                gate_dv(a)
                if B:
                    B[0]()
                gate_dv(b)
                lvlm(a, 32)
                pa_dv(a)
                if B:
                    B[1]()
                    B[2]()
                pa_dv(b)
                lvlm(a, 16)
                x0(a)
                if B:
                    B[3]()
                    B[4]()
                x0(b)
                lvlm(a, 8)
                if B:
                    B[5]()
                    B[6]()
                redm(a)
                if B:
                    B[7]()
                    B[8]()
                    pe(a - 2)
                    pe(b - 2)
            # drain: Pool mult + blend + PE for the last pair
            pa_gp(n_ch - 2)
            pa_gp(n_ch - 1)
            for th in blend_thunks(base[n_ch - 2] // C, S_TOT):
                th()
            pe(n_ch - 2)
            pe(n_ch - 1)

            # output: per-partition sum of term2 plus the psum column
            res0 = stat_pool.tile([128, 1], f32, name="res0")
            nc.vector.tensor_reduce(out=res0[:], in_=term2[:], axis=X, op=A.add)
            res = stat_pool.tile([128, 1], f32, name="res")
            nc.vector.tensor_tensor(out=res[:], in0=res0[:], in1=psum[:], op=A.add)
            nc.sync.dma_start(out=out_d[:, :], in_=res[:])

    nc.compile()
    return nc


def _get_nc():
    if "nc" not in _cache:
        _cache["nc"] = _build_nc()
    return _cache["nc"]


def _encode(x: np.ndarray, target: np.ndarray) -> np.ndarray:
    """bf16(x) rounded to even-LSB grid, with target's bit in the LSB."""
    bits = np.ascontiguousarray(x, dtype=np.float32).view(np.uint32)
    t = target.astype(np.uint32)
    # shift magnitude down by t * 1ulp16 before rounding so the +t*ulp16 of
    # the LSB write-back is centered (removes the ~0.4% upward bias on fs)
    bits = bits - (t << np.uint32(16))
    b15 = (bits + np.uint32(0xFFFF) + ((bits >> np.uint32(17)) & 1)) >> np.uint32(17)
    return ((b15 << np.uint32(1)) | t).astype(np.uint16)


def kernel(x: np.ndarray, target: np.ndarray) -> np.ndarray:
    from concourse.bass_utils import run_bass_kernel_spmd

    nc = _get_nc()
    enc = _encode(np.asarray(x), np.asarray(target))
    es = enc.reshape(N_CORES, ROWS_PER_CORE, C)
    in_maps = [{"x": es[i]} for i in range(N_CORES)]
    r = run_bass_kernel_spmd(nc, in_maps, core_ids=list(range(N_CORES)))
    total = np.float64(0.0)
    for res in r.results:
        total += np.sum(res["out"].astype(np.float64))
    return np.float32(total / (N * C))
